# revision 3
# baseline (speedup 1.0000x reference)
"""Trainium2 Bass kernel v4 for nn_CrossAttention — N=512 moving operands.

Data-parallel over B=8 (one batch element per core, no collectives). K^T and V
are projected once into resident SBUF; attention runs in q-chunks of 512 with
fp32 PSUM accumulation. All matmuls use 512-wide moving operands in >=8-MM
accumulation groups (one PSUM start-clear per group), halving the instruction
stream vs 256-wide tiling. PSUM: qproj/rowsum 1 bank, QK scores 3 rotating
banks, PV accumulators 4 banks (j-major, two H-half passes).
"""

import sys

sys.path.insert(0, "/opt/trn_rl_repo")

import numpy as np
import ml_dtypes

B, T, C, H = 8, 4096, 1024, 1024
NCORES = 8

CT = C // 128   # 8 contraction tiles
HT = H // 128   # 8 h tiles
ST = T // 128   # 32 key tiles
TCH = 512       # projection t-chunk
NTCH = T // TCH
QCH = 512       # attention q-chunk
NQCH = T // QCH
QS = QCH // 128  # q sub-tiles per chunk (4)

_CACHE = {}


def _build(reps=1, loop=False):
    import concourse.bacc as bacc
    import concourse.tile as tile
    from concourse import mybir

    f32 = mybir.dt.float32
    bf16 = mybir.dt.bfloat16

    nc = bacc.Bacc("TRN2", target_bir_lowering=False, debug=False,
                   num_devices=NCORES)

    xT = nc.dram_tensor("xT", [C, T], bf16, kind="ExternalInput").ap()
    wqT = nc.dram_tensor("wqT", [C, H], bf16, kind="ExternalInput").ap()
    wkT = nc.dram_tensor("wkT", [C, H], bf16, kind="ExternalInput").ap()
    wvT = nc.dram_tensor("wvT", [C, H], bf16, kind="ExternalInput").ap()
    out = nc.dram_tensor("out", [T, H], f32, kind="ExternalOutput").ap()

    xTr = xT.rearrange("(a p) t -> p a t", p=128)
    wqr = wqT.rearrange("(a p) h -> p a h", p=128)
    wkr = wkT.rearrange("(a p) h -> p a h", p=128)
    wvr = wvT.rearrange("(a p) h -> p a h", p=128)

    scale = 1.0 / np.sqrt(np.float32(C))

    with tile.TileContext(nc) as tc:
        with tc.tile_pool(name="singles", bufs=1) as singles, \
             tc.tile_pool(name="wsb", bufs=1) as wsb, \
             tc.tile_pool(name="xp", bufs=2) as xp, \
             tc.tile_pool(name="qtp", bufs=1) as qtp, \
             tc.tile_pool(name="ptp", bufs=32) as ptp, \
             tc.tile_pool(name="accp", bufs=1) as accp, \
             tc.tile_pool(name="rcp", bufs=8) as rcp, \
             tc.tile_pool(name="op", bufs=1) as op, \
             tc.tile_pool(name="wpre", bufs=1) as wpre, \
             tc.tile_pool(name="pss", bufs=4, space="PSUM") as pss, \
             tc.tile_pool(name="pvp", bufs=4, space="PSUM") as pvp:

            kt_sb = singles.tile([128, HT, T], bf16, tag="kt")    # K^T [h, t]
            v_sb = singles.tile([128, ST, H], bf16, tag="v")      # V [s, h]
            ones = singles.tile([128, 1], f32, tag="ones")
            nc.vector.memset(ones, 1.0)

            def emit_rep(rep):
                # ---- Phase 1a: K^T = Wk @ xT into resident SBUF ----
                # interleave wk/xt0 slice loads: first MM needs wk[c0]+xt0[c0]
                wk = wsb.tile([128, CT, H], bf16, tag="w", name=f"wk{rep}")
                xt0 = xp.tile([128, CT, TCH], bf16, tag="x",
                              name=f"xtk{rep}_0")
                for c_ in range(CT):
                    nc.sync.dma_start(out=wk[:, c_, :], in_=wkr[:, c_, :])
                    nc.sync.dma_start(out=xt0[:, c_, :],
                                      in_=xTr[:, c_, 0:TCH])
                wpre_v = wpre.tile([128, 1, H], bf16, tag="wp",
                                   name=f"wpv{rep}")
                nc.sync.dma_start(out=wpre_v, in_=wvr[:, 0:1, :])
                for tch in range(NTCH):
                    t0 = tch * TCH
                    if tch == 0:
                        xt = xt0
                    else:
                        xt = xp.tile([128, CT, TCH], bf16, tag="x",
                                     name=f"xtk{rep}_{tch}")
                        nc.sync.dma_start(out=xt, in_=xTr[:, :, t0:t0 + TCH])
                    if tch == NTCH - 1:
                        xt_lastk = xt
                    for h in range(HT):
                        ps = pss.tile([128, TCH], f32, tag="s",
                                      name=f"psk{rep}_{tch}_{h}")
                        for c in range(CT):
                            nc.tensor.matmul(
                                ps, wk[:, c, h * 128:(h + 1) * 128],
                                xt[:, c, :],
                                start=(c == 0), stop=(c == CT - 1),
                                skip_group_check=True)
                        nc.scalar.copy(out=kt_sb[:, h, t0:t0 + TCH], in_=ps)

                # ---- Phase 1b: V = x @ Wv^T into resident SBUF ----
                # tch descending: tch=NTCH-1 reuses the still-resident last
                # K-pass x tile, so the phase switch pays no DMA wait; the
                # same trick hands the tch=0 tile to q-chunk 0's projection.
                wv = wsb.tile([128, CT, H], bf16, tag="w", name=f"wv{rep}")
                for c_ in range(CT):
                    nc.sync.dma_start(out=wv[:, c_, :], in_=wvr[:, c_, :])
                wv0 = wpre_v  # c=0 slice, prefetched during phase 1a
                wpre_q = wpre.tile([128, 1, H], bf16, tag="wp",
                                   name=f"wpq{rep}")
                nc.sync.dma_start(out=wpre_q, in_=wqr[:, 0:1, :])
                for tch in range(NTCH - 1, -1, -1):
                    t0 = tch * TCH
                    if tch == NTCH - 1:
                        xt = xt_lastk
                    else:
                        xt = xp.tile([128, CT, TCH], bf16, tag="x",
                                     name=f"xtv{rep}_{tch}")
                        nc.sync.dma_start(out=xt, in_=xTr[:, :, t0:t0 + TCH])
                    if tch == 0:
                        xt_first = xt
                    for ts in range(TCH // 128):
                        s_idx = tch * (TCH // 128) + ts
                        pv0 = pvp.tile([128, 512], f32, tag="o",
                                       name=f"psv{rep}_{tch}_{ts}_0")
                        pv1 = pvp.tile([128, 512], f32, tag="o",
                                       name=f"psv{rep}_{tch}_{ts}_1")
                        pvs = (pv0, pv1)
                        for c in range(CT):
                            wsrc = wv0[:, 0, :] if c == 0 else wv[:, c, :]
                            for hc in range(2):
                                nc.tensor.matmul(
                                    pvs[hc],
                                    xt[:, c, ts * 128:(ts + 1) * 128],
                                    wsrc[:, hc * 512:(hc + 1) * 512],
                                    start=(c == 0), stop=(c == CT - 1),
                                    skip_group_check=True)
                        for hc in range(2):
                            nc.vector.tensor_copy(
                                out=v_sb[:, s_idx, hc * 512:(hc + 1) * 512],
                                in_=pvs[hc])

                # ---- Phase 2: attention, q-chunks of QCH=512 ----
                wq = wsb.tile([128, CT, H], bf16, tag="w", name=f"wq{rep}")
                for c_ in range(CT):
                    nc.sync.dma_start(out=wq[:, c_, :], in_=wqr[:, c_, :])
                wq0 = wpre_q  # c=0 slice, prefetched during phase 1b

                def emit_qtproj(qch):
                    q0 = qch * QCH
                    if qch == 0:
                        xq = xt_first
                    else:
                        xq = xp.tile([128, CT, QCH], bf16, tag="x",
                                     name=f"xq{rep}_{qch}")
                        nc.sync.dma_start(out=xq, in_=xTr[:, :, q0:q0 + QCH])
                    qt = qtp.tile([128, HT, QCH], bf16, tag="qt",
                                  name=f"qt{rep}_{qch}")
                    for h in range(HT):
                        ps = pss.tile([128, QCH], f32, tag="s",
                                      name=f"psq{rep}_{qch}_{h}")
                        for c in range(CT):
                            wsrc = wq0[:, 0, :] if (qch == 0 and c == 0) \
                                else wq[:, c, :]
                            nc.tensor.matmul(ps,
                                             wsrc[:, h * 128:(h + 1) * 128],
                                             xq[:, c, :],
                                             start=(c == 0), stop=(c == CT - 1),
                                             skip_group_check=True)
                        nc.scalar.copy(out=qt[:, h, :], in_=ps)
                    return qt

                qt_next = emit_qtproj(0)
                for qch in range(NQCH):
                    q0 = qch * QCH
                    qt = qt_next
                    # scores S^T[s, q] = K^T.T @ Q^T ; P = exp(S * scale)
                    acc = accp.tile([128, QCH], f32, tag="acc",
                                    name=f"acc{rep}_{qch}")
                    pts = []
                    for s in range(ST):
                        ps = pss.tile([128, QCH], f32, tag="s",
                                      name=f"pss{rep}_{qch}_{s}")
                        for h in range(HT):
                            nc.tensor.matmul(ps,
                                             kt_sb[:, h, s * 128:(s + 1) * 128],
                                             qt[:, h, :],
                                             start=(h == 0), stop=(h == HT - 1),
                                             skip_group_check=True)
                        pt = ptp.tile([128, QCH], bf16, tag="pt",
                                      name=f"pt{rep}_{qch}_{s}")
                        nc.scalar.activation(out=pt, in_=ps,
                                             func=mybir.ActivationFunctionType.Exp,
                                             scale=float(scale))
                        pts.append(pt)
                        if s == 0:
                            nc.vector.tensor_copy(out=acc, in_=pt)
                        else:
                            nc.vector.tensor_add(out=acc, in0=acc, in1=pt)
                    # hoisted Q^T projection for the next chunk: its PE work
                    # lands between QK and PV so ACT copies overlap PV
                    if qch + 1 < NQCH:
                        qt_next = emit_qtproj(qch + 1)
                    # rowsum -> reciprocal (DVE add chain finishes during the
                    # hoisted projection; recips ready before PV drains)
                    recips = []
                    for j in range(QS):
                        psr = pss.tile([128, 1], f32, tag="s",
                                       name=f"psr{rep}_{qch}_{j}")
                        nc.tensor.matmul(psr, acc[:, j * 128:(j + 1) * 128],
                                         ones, start=True, stop=True,
                                         skip_group_check=True)
                        rc = rcp.tile([128, 1], f32, tag="rc",
                                      name=f"rc{rep}_{qch}_{j}")
                        nc.vector.reciprocal(out=rc, in_=psr)
                        recips.append(rc)
                    # O = P^T.T @ V, two H-half passes, j-major so each
                    # accumulator drains while the next one fills
                    for hc in range(2):
                        for j in range(QS):
                            po = pvp.tile([128, 512], f32, tag="o",
                                          name=f"po{rep}_{qch}_{hc}_{j}")
                            for s in range(ST):
                                nc.tensor.matmul(
                                    po,
                                    pts[s][:, j * 128:(j + 1) * 128],
                                    v_sb[:, s, hc * 512:(hc + 1) * 512],
                                    start=(s == 0), stop=(s == ST - 1),
                                    skip_group_check=True)
                            ob = op.tile([128, 512], f32, tag="ob",
                                         name=f"ob{rep}_{qch}_{hc}_{j}")
                            nc.vector.tensor_scalar_mul(ob, po, recips[j])
                            nc.sync.dma_start(
                                out=out[q0 + j * 128:q0 + (j + 1) * 128,
                                        hc * 512:(hc + 1) * 512],
                                in_=ob)

            if loop and reps > 1:
                from concourse import mybir as _mb
                engs = [_mb.EngineType.PE, _mb.EngineType.Activation,
                        _mb.EngineType.DVE, _mb.EngineType.SP]
                with tc.For_i(0, reps, 1, hint_engines=tuple(engs)):
                    emit_rep(0)
            else:
                for rep in range(reps):
                    emit_rep(rep)

    nc.compile()
    return nc


def _get_program(reps=1):
    if reps not in _CACHE:
        _CACHE[reps] = _build(reps)
    return _CACHE[reps]


def prep_inputs(x, Wq, Wk, Wv):
    """Host-side shard + layout prep: returns per-core input maps."""
    x = np.asarray(x, dtype=np.float32)
    bf = ml_dtypes.bfloat16
    wqT = np.ascontiguousarray(np.asarray(Wq, dtype=np.float32).T).astype(bf)
    wkT = np.ascontiguousarray(np.asarray(Wk, dtype=np.float32).T).astype(bf)
    wvT = np.ascontiguousarray(np.asarray(Wv, dtype=np.float32).T).astype(bf)
    in_maps = []
    for b in range(NCORES):
        xTb = np.ascontiguousarray(x[b].T).astype(bf)
        in_maps.append({"xT": xTb, "wqT": wqT, "wkT": wkT, "wvT": wvT})
    return in_maps


def kernel(x, Wq, Wk, Wv):
    from concourse import bass_utils

    in_maps = prep_inputs(x, Wq, Wk, Wv)
    nc = _get_program(reps=1)
    res = bass_utils.run_bass_kernel_spmd(nc, in_maps, list(range(NCORES)))
    return np.stack([res.results[c]["out"] for c in range(NCORES)], axis=0)
